# revision 41
# baseline (speedup 1.0000x reference)
"""Trainium2 Bass kernel for nn_Expansion_55044300865687.

The reference module is linear in x_in: per-irrep channel sums (3200 -> 25)
followed by a constant Wigner-3j expansion matrix A2 (25 -> 196 = 14*14).
Memory-bound: the 640 MB input read dominates (~238 us/core HBM floor).

Host side: batch-shard x over 8 cores; permute columns per irrep to
[m][channel] order so the channel axis is unit-stride for the DVE reduce.

Per core (6250 rows = 48 full 128-row tiles + one 106-row tail):
  - x loads in 6.4 MB pool slots filled by four back-to-back 1.6 MB
    per-tile DMAs (deep SDMA descriptor queues -> full memory-level
    parallelism, while each tile's reduce waits only for its own bytes)
  - DVE: 5 unit-stride reduce_sum ops (channel sum per irrep) -> s [128, 25]
  - PE:  transpose s -> PSUM [25, 128], ACT copy -> SBUF
  - PE:  matmul lhsT=sT [25,nb], rhs=A2 [25,196] -> PSUM [nb,196], ACT copy
  - 7 tiles of output buffered in SBUF, one 1.37 MB out-DMA per group
    (gpsimd/SWDGE, off the HWDGE rings feeding the x stream)

Output DRAM layout per core is [128, 49, 196] (partition-major tiles) so
out-DMA runs are long; the host reassembles to [6250, 196].
"""

import math
from contextlib import ExitStack
from fractions import Fraction

import numpy as np

import concourse.bacc as bacc
import concourse.bass as bass  # noqa: F401  (AP helpers)
import concourse.mybir as mybir
import concourse.tile as tile
from concourse.bass_utils import run_bass_kernel_spmd

# ---------------------------------------------------------------- problem dims
N_CORES = 8
BATCH = 50000
ROWS = BATCH // N_CORES            # 6250 rows per core
DIM_IN = 3200
P = 128
N_TILES = (ROWS + P - 1) // P      # 49 (48 full + 106-row tail)
TAIL = ROWS - (N_TILES - 1) * P    # 106
OUT_COLS = 196                     # 14*14
GROUP = 7                          # output tiles buffered per out-DMA (49 = 7*7)

IRREP_IN = [(128, 0, 1), (128, 1, -1), (128, 2, 1), (128, 3, -1), (128, 4, 1)]
IRREP_OUT1 = [(3, 0, 1), (2, 1, -1), (1, 2, 1)]
IRREP_OUT2 = [(3, 0, 1), (2, 1, -1), (1, 2, 1)]
D_IN = [2 * l + 1 for _, l, _ in IRREP_IN]         # [1,3,5,7,9]
S_DIM = sum(D_IN)                                   # 25

# Host column permutation: within each irrep block, store [m][channel]
# (channel innermost, unit stride) instead of the reference's [channel][m].
# FOLD > 1 would additionally split channels into groups accumulated during
# DMA (SWDGE accum_op) — measured a net loss on TRN2 (accumulating DMAs run
# at ~half rate per byte), so FOLD stays 1.
FOLD = 1
CH = 128 // FOLD                   # channels per group
W = DIM_IN // FOLD                 # permuted columns per group


def _build_perm():
    """PERM[new_col] = old_col for the host-side column permutation."""
    perm = np.empty(DIM_IN, dtype=np.int64)
    pos = 0
    for g in range(FOLD):
        off = 0
        for (mul, l, _), d in zip(IRREP_IN, D_IN):
            for m in range(d):
                for cp in range(CH):
                    perm[pos] = off + (g * CH + cp) * d + m
                    pos += 1
            off += mul * d
    return perm


PERM = _build_perm()


def _permute(x):
    return np.ascontiguousarray(x[:, PERM])


# ------------------------------------------------------- Wigner-3j constants
def _su2_cg(j1, m1, j2, m2, j3, m3):
    if m3 != m1 + m2:
        return 0.0
    f = math.factorial
    vmin = max(-j1 + j2 + m3, -j1 + m1, 0)
    vmax = min(j2 + j3 + m1, j3 - j1 + j2, j3 + m3)
    C = math.sqrt((2 * j3 + 1) * Fraction(
        f(j3 + j1 - j2) * f(j3 - j1 + j2) * f(j1 + j2 - j3) * f(j3 + m3) * f(j3 - m3),
        f(j1 + j2 + j3 + 1) * f(j1 - m1) * f(j1 + m1) * f(j2 - m2) * f(j2 + m2)))
    S = Fraction(0)
    for v in range(vmin, vmax + 1):
        S += (-1) ** (v + j2 + m2) * Fraction(
            f(j2 + j3 + m1 - v) * f(j1 - m1 + v),
            f(v) * f(j3 - j1 + j2 - v) * f(j3 + m3 - v) * f(v + j1 - j2 - m3))
    return C * float(S)


def _q(l):
    q = np.zeros((2 * l + 1, 2 * l + 1), dtype=np.complex128)
    for m in range(-l, 0):
        q[l + m, l + abs(m)] = 1 / math.sqrt(2)
        q[l + m, l - abs(m)] = -1j / math.sqrt(2)
    q[l, l] = 1.0
    for m in range(1, l + 1):
        q[l + m, l + abs(m)] = (-1) ** m / math.sqrt(2)
        q[l + m, l - abs(m)] = 1j * (-1) ** m / math.sqrt(2)
    return (-1j) ** l * q


def _wigner_3j(l1, l2, l3):
    C = np.zeros((2 * l1 + 1, 2 * l2 + 1, 2 * l3 + 1))
    for i, m1 in enumerate(range(-l1, l1 + 1)):
        for j, m2 in enumerate(range(-l2, l2 + 1)):
            for k, m3 in enumerate(range(-l3, l3 + 1)):
                C[i, j, k] = _su2_cg(l1, m1, l2, m2, l3, m3)
    C = np.einsum('ij,kl,mn,ikn->jlm', _q(l1), _q(l2), np.conj(_q(l3).T),
                  C.astype(np.complex128))
    C = np.real(C)
    return (C / np.linalg.norm(C)).astype(np.float32)


def _build_a2():
    """A2 [25, 196]: maps per-irrep channel sums to the flattened 14x14 output."""
    m_off = np.concatenate([[0], np.cumsum(D_IN)])
    row_off = np.concatenate(
        [[0], np.cumsum([u * (2 * l1 + 1) for u, l1, _ in IRREP_OUT1])])
    col_off = np.concatenate(
        [[0], np.cumsum([v * (2 * l2 + 1) for v, l2, _ in IRREP_OUT2])])
    a2 = np.zeros((S_DIM, OUT_COLS), dtype=np.float64)
    for i, (_, li, pi) in enumerate(IRREP_IN):
        for j, (u, l1, p1) in enumerate(IRREP_OUT1):
            for k, (v, l2, p2) in enumerate(IRREP_OUT2):
                if abs(l1 - l2) <= li <= l1 + l2 and pi == p1 * p2:
                    d1, d2 = 2 * l1 + 1, 2 * l2 + 1
                    w = _wigner_3j(l1, l2, li).astype(np.float64)  # [d1,d2,d_i]
                    for a in range(u):
                        for b in range(v):
                            for p in range(d1):
                                for q in range(d2):
                                    r = row_off[j] + a * d1 + p
                                    c = col_off[k] + b * d2 + q
                                    a2[m_off[i]:m_off[i] + D_IN[i], r * 14 + c] += \
                                        w[p, q, :]
    return a2.astype(np.float32)


# ---------------------------------------------------------------- bass module
def _build_nc(rows=ROWS):
    f32 = mybir.dt.float32
    n_tiles = (rows + P - 1) // P

    nc = bacc.Bacc(None, target_bir_lowering=False)
    x = nc.declare_dram_parameter("x", [rows, DIM_IN], f32, isOutput=False)
    out = nc.declare_dram_parameter("out", [P, n_tiles, OUT_COLS], f32,
                                    isOutput=True)
    a2_dram = nc.inline_tensor(_build_a2(), name="a2")
    id_dram = nc.inline_tensor(np.eye(P, dtype=np.float32), name="ident")

    # per-irrep (column offset in the folded SBUF tile, 2l+1, offset in s)
    irrep_meta = []
    boff = 0
    moff = 0
    for d in D_IN:
        irrep_meta.append((boff, d, moff))
        boff += d * CH
        moff += d

    TPB = 4  # batch-tiles loaded per x-DMA (6.4 MB per transfer)

    with tile.TileContext(nc) as tc, ExitStack() as ctx:
        consts = ctx.enter_context(tc.tile_pool(name="consts", bufs=1))
        xpool = ctx.enter_context(tc.tile_pool(name="xp", bufs=2))
        xtailp = ctx.enter_context(tc.tile_pool(name="xtail", bufs=1))
        spool = ctx.enter_context(tc.tile_pool(name="sp", bufs=4))
        stpool = ctx.enter_context(tc.tile_pool(name="stp", bufs=4))
        opool = ctx.enter_context(tc.tile_pool(name="op", bufs=4))
        ps_t = ctx.enter_context(tc.tile_pool(name="ps_t", bufs=2, space="PSUM"))
        ps_o = ctx.enter_context(tc.tile_pool(name="ps_o", bufs=2, space="PSUM"))

        a2_sb = consts.tile([S_DIM, OUT_COLS], f32)
        nc.sync.dma_start(out=a2_sb[:], in_=a2_dram[:])
        id_sb = consts.tile([P, P], f32)
        nc.sync.dma_start(out=id_sb[:], in_=id_dram[:])

        # ---- burst x loads: up to TPB full 128-row tiles per DMA ----
        n_full = rows // P                       # full 128-row tiles (48)
        bursts = []                              # (first_tile, n_subtiles)
        t0 = 0
        while t0 < n_full:
            ntb = min(TPB, n_full - t0)
            bursts.append((t0, ntb))
            t0 += ntb

        xt_of = {}                               # tile idx -> (xt, col offset)

        def load_burst(t0, ntb):
            # One pool slot per burst, but issue per-tile sub-DMAs back to
            # back: the SDMA engines get deep descriptor queues (full memory-
            # level parallelism) while each tile's reduce only waits for its
            # own 1.6 MB.
            xt = xpool.tile([P, TPB * W], f32)
            for j in range(ntb):
                nc.sync.dma_start(out=xt[:, j * W:(j + 1) * W],
                                  in_=x[(t0 + j) * P:(t0 + j + 1) * P, :])
                xt_of[t0 + j] = (xt, j * W)

        def load_tail():
            xt = xtailp.tile([P, W], f32)
            r0 = n_full * P
            nc.sync.dma_start(out=xt[:rows - r0, :], in_=x[r0:rows, :])
            xt_of[n_full] = (xt, 0)

        next_burst = {t0: (t0, ntb) for (t0, ntb) in bursts}

        # output groups of GROUP tiles, but keep the ragged tail tile in its
        # own final group so the last (exposed) out-DMA is tiny
        group_of = {}
        g = 0
        while g < n_full:
            ge = min(g + GROUP, n_full)
            group_of[g] = (g, ge)
            g = ge
        if n_full < n_tiles:
            group_of[n_full] = (n_full, n_tiles)

        for t in range(n_tiles):
            r0 = t * P
            nb = min(P, rows - r0)

            if t in next_burst:
                load_burst(*next_burst[t])
            if t == n_full:
                load_tail()

            if t in group_of:
                g0, g1 = group_of[t]
                out_sb = opool.tile([P, GROUP * OUT_COLS], f32)
            ocols = slice((t - g0) * OUT_COLS, (t - g0 + 1) * OUT_COLS)

            if nb < P:
                # tail tile: zero the block so partitions [nb:] hold zeros
                nc.vector.memset(out_sb[:, ocols], 0.0)

            xt, xoff = xt_of.pop(t)
            st = spool.tile([P, S_DIM], f32)
            # permuted layout is uniformly [25 m-slots][128 channels], so the
            # whole channel-sum is ONE unit-stride reduce over [p, 25, 128]
            src = xt[:nb, xoff:xoff + W].rearrange("p (m c) -> p m c", c=CH)
            nc.vector.reduce_sum(out=st[:nb, :], in_=src,
                                 axis=mybir.AxisListType.X)

            pT = ps_t.tile([S_DIM, P], f32)
            nc.tensor.transpose(out=pT[:, :nb], in_=st[:nb, :],
                                identity=id_sb[:nb, :nb])
            sT = stpool.tile([S_DIM, P], f32)
            nc.scalar.copy(out=sT[:, :nb], in_=pT[:, :nb])

            po = ps_o.tile([P, OUT_COLS], f32)
            nc.tensor.matmul(out=po[:nb, :], lhsT=sT[:, :nb], rhs=a2_sb[:],
                             start=True, stop=True)
            nc.scalar.copy(out=out_sb[:nb, ocols], in_=po[:nb, :])

            if t == g1 - 1:
                nc.gpsimd.dma_start(
                    out=out[:, g0:g1, :],
                    in_=out_sb[:, 0:(g1 - g0) * OUT_COLS].rearrange(
                        "p (t c) -> p t c", c=OUT_COLS))
    nc.finalize()
    return nc


_NC_CACHE = {}


def _get_nc(rows=ROWS):
    if rows not in _NC_CACHE:
        _NC_CACHE[rows] = _build_nc(rows)
    return _NC_CACHE[rows]


LAST_RESULTS = None  # BassKernelResults of the most recent kernel() call


def kernel(x_in: np.ndarray) -> np.ndarray:
    global LAST_RESULTS
    x = np.ascontiguousarray(np.asarray(x_in), dtype=np.float32)
    assert x.shape == (BATCH, DIM_IN), x.shape
    nc = _get_nc()
    xp = _permute(x)
    in_maps = [{"x": xp[c * ROWS:(c + 1) * ROWS]} for c in range(N_CORES)]
    res = run_bass_kernel_spmd(nc, in_maps, core_ids=list(range(N_CORES)))
    LAST_RESULTS = res
    outs = []
    for c in range(N_CORES):
        o = res.results[c]["out"]                       # [128, 49, 196]
        o = o.transpose(1, 0, 2).reshape(N_TILES * P, OUT_COLS)[:ROWS]
        outs.append(o)
    return np.concatenate(outs, axis=0).reshape(BATCH, 14, 14)


# revision 44
# speedup vs baseline: 1.1160x; 1.1160x over previous
"""Trainium2 Bass kernel for nn_Expansion_55044300865687.

The reference module is linear in x_in: per-irrep channel sums (3200 -> 25)
followed by a constant Wigner-3j expansion matrix A2 (25 -> 196 = 14*14).
Memory-bound: the 640 MB input read dominates (~238 us/core HBM floor).

Host side: batch-shard x over 8 cores; permute columns per irrep to
[m][channel] order so the channel axis is unit-stride for the DVE reduce.

Per core (6250 rows = 48 full 128-row tiles + one 106-row tail):
  - x loads in 6.4 MB pool slots filled by four back-to-back 1.6 MB
    per-tile DMAs (deep SDMA descriptor queues -> full memory-level
    parallelism, while each tile's reduce waits only for its own bytes)
  - DVE: 5 unit-stride reduce_sum ops (channel sum per irrep) -> s [128, 25]
  - PE:  transpose s -> PSUM [25, 128], ACT copy -> SBUF
  - PE:  matmul lhsT=sT [25,nb], rhs=A2 [25,196] -> PSUM [nb,196], ACT copy
  - 7 tiles of output buffered in SBUF, one 1.37 MB out-DMA per group
    (gpsimd/SWDGE, off the HWDGE rings feeding the x stream)

Output DRAM layout per core is [128, 49, 196] (partition-major tiles) so
out-DMA runs are long; the host reassembles to [6250, 196].
"""

import math
from contextlib import ExitStack
from fractions import Fraction

import numpy as np

import concourse.bacc as bacc
import concourse.bass as bass  # noqa: F401  (AP helpers)
import concourse.mybir as mybir
import concourse.tile as tile
from concourse.bass_utils import run_bass_kernel_spmd

# ---------------------------------------------------------------- problem dims
N_CORES = 8
BATCH = 50000
ROWS = BATCH // N_CORES            # 6250 rows per core
DIM_IN = 3200
P = 128
N_TILES = (ROWS + P - 1) // P      # 49 (48 full + 106-row tail)
TAIL = ROWS - (N_TILES - 1) * P    # 106
OUT_COLS = 196                     # 14*14
GROUP = 7                          # output tiles buffered per out-DMA (49 = 7*7)

IRREP_IN = [(128, 0, 1), (128, 1, -1), (128, 2, 1), (128, 3, -1), (128, 4, 1)]
IRREP_OUT1 = [(3, 0, 1), (2, 1, -1), (1, 2, 1)]
IRREP_OUT2 = [(3, 0, 1), (2, 1, -1), (1, 2, 1)]
D_IN = [2 * l + 1 for _, l, _ in IRREP_IN]         # [1,3,5,7,9]
S_DIM = sum(D_IN)                                   # 25

# Host column permutation: within each irrep block, store [m][channel]
# (channel innermost, unit stride) instead of the reference's [channel][m].
# FOLD > 1 would additionally split channels into groups accumulated during
# DMA (SWDGE accum_op) — measured a net loss on TRN2 (accumulating DMAs run
# at ~half rate per byte), so FOLD stays 1.
FOLD = 1
CH = 128 // FOLD                   # channels per group
W = DIM_IN // FOLD                 # permuted columns per group


def _build_perm():
    """PERM[new_col] = old_col for the host-side column permutation."""
    perm = np.empty(DIM_IN, dtype=np.int64)
    pos = 0
    for g in range(FOLD):
        off = 0
        for (mul, l, _), d in zip(IRREP_IN, D_IN):
            for m in range(d):
                for cp in range(CH):
                    perm[pos] = off + (g * CH + cp) * d + m
                    pos += 1
            off += mul * d
    return perm


PERM = _build_perm()


def _permute(x):
    return np.ascontiguousarray(x[:, PERM])


# ------------------------------------------------------- Wigner-3j constants
def _su2_cg(j1, m1, j2, m2, j3, m3):
    if m3 != m1 + m2:
        return 0.0
    f = math.factorial
    vmin = max(-j1 + j2 + m3, -j1 + m1, 0)
    vmax = min(j2 + j3 + m1, j3 - j1 + j2, j3 + m3)
    C = math.sqrt((2 * j3 + 1) * Fraction(
        f(j3 + j1 - j2) * f(j3 - j1 + j2) * f(j1 + j2 - j3) * f(j3 + m3) * f(j3 - m3),
        f(j1 + j2 + j3 + 1) * f(j1 - m1) * f(j1 + m1) * f(j2 - m2) * f(j2 + m2)))
    S = Fraction(0)
    for v in range(vmin, vmax + 1):
        S += (-1) ** (v + j2 + m2) * Fraction(
            f(j2 + j3 + m1 - v) * f(j1 - m1 + v),
            f(v) * f(j3 - j1 + j2 - v) * f(j3 + m3 - v) * f(v + j1 - j2 - m3))
    return C * float(S)


def _q(l):
    q = np.zeros((2 * l + 1, 2 * l + 1), dtype=np.complex128)
    for m in range(-l, 0):
        q[l + m, l + abs(m)] = 1 / math.sqrt(2)
        q[l + m, l - abs(m)] = -1j / math.sqrt(2)
    q[l, l] = 1.0
    for m in range(1, l + 1):
        q[l + m, l + abs(m)] = (-1) ** m / math.sqrt(2)
        q[l + m, l - abs(m)] = 1j * (-1) ** m / math.sqrt(2)
    return (-1j) ** l * q


def _wigner_3j(l1, l2, l3):
    C = np.zeros((2 * l1 + 1, 2 * l2 + 1, 2 * l3 + 1))
    for i, m1 in enumerate(range(-l1, l1 + 1)):
        for j, m2 in enumerate(range(-l2, l2 + 1)):
            for k, m3 in enumerate(range(-l3, l3 + 1)):
                C[i, j, k] = _su2_cg(l1, m1, l2, m2, l3, m3)
    C = np.einsum('ij,kl,mn,ikn->jlm', _q(l1), _q(l2), np.conj(_q(l3).T),
                  C.astype(np.complex128))
    C = np.real(C)
    return (C / np.linalg.norm(C)).astype(np.float32)


def _build_a2():
    """A2 [25, 196]: maps per-irrep channel sums to the flattened 14x14 output."""
    m_off = np.concatenate([[0], np.cumsum(D_IN)])
    row_off = np.concatenate(
        [[0], np.cumsum([u * (2 * l1 + 1) for u, l1, _ in IRREP_OUT1])])
    col_off = np.concatenate(
        [[0], np.cumsum([v * (2 * l2 + 1) for v, l2, _ in IRREP_OUT2])])
    a2 = np.zeros((S_DIM, OUT_COLS), dtype=np.float64)
    for i, (_, li, pi) in enumerate(IRREP_IN):
        for j, (u, l1, p1) in enumerate(IRREP_OUT1):
            for k, (v, l2, p2) in enumerate(IRREP_OUT2):
                if abs(l1 - l2) <= li <= l1 + l2 and pi == p1 * p2:
                    d1, d2 = 2 * l1 + 1, 2 * l2 + 1
                    w = _wigner_3j(l1, l2, li).astype(np.float64)  # [d1,d2,d_i]
                    for a in range(u):
                        for b in range(v):
                            for p in range(d1):
                                for q in range(d2):
                                    r = row_off[j] + a * d1 + p
                                    c = col_off[k] + b * d2 + q
                                    a2[m_off[i]:m_off[i] + D_IN[i], r * 14 + c] += \
                                        w[p, q, :]
    return a2.astype(np.float32)


# ---------------------------------------------------------------- bass module
def _build_nc(rows=ROWS):
    f32 = mybir.dt.float32
    n_tiles = (rows + P - 1) // P

    nc = bacc.Bacc(None, target_bir_lowering=False)
    x = nc.declare_dram_parameter("x", [rows, DIM_IN], f32, isOutput=False)
    out = nc.declare_dram_parameter("out", [P, n_tiles, OUT_COLS], f32,
                                    isOutput=True)
    a2_dram = nc.inline_tensor(_build_a2(), name="a2")
    id_dram = nc.inline_tensor(np.eye(P, dtype=np.float32), name="ident")

    # per-irrep (column offset in the folded SBUF tile, 2l+1, offset in s)
    irrep_meta = []
    boff = 0
    moff = 0
    for d in D_IN:
        irrep_meta.append((boff, d, moff))
        boff += d * CH
        moff += d

    TPB = 4  # batch-tiles loaded per x-DMA (6.4 MB per transfer)

    with tile.TileContext(nc) as tc, ExitStack() as ctx:
        consts = ctx.enter_context(tc.tile_pool(name="consts", bufs=1))
        xpool = ctx.enter_context(tc.tile_pool(name="xp", bufs=2))
        xtailp = ctx.enter_context(tc.tile_pool(name="xtail", bufs=1))
        spool = ctx.enter_context(tc.tile_pool(name="sp", bufs=4))
        stpool = ctx.enter_context(tc.tile_pool(name="stp", bufs=4))
        opool = ctx.enter_context(tc.tile_pool(name="op", bufs=4))
        ps_t = ctx.enter_context(tc.tile_pool(name="ps_t", bufs=2, space="PSUM"))
        ps_o = ctx.enter_context(tc.tile_pool(name="ps_o", bufs=2, space="PSUM"))

        a2_sb = consts.tile([S_DIM, OUT_COLS], f32)
        nc.sync.dma_start(out=a2_sb[:], in_=a2_dram[:])
        id_sb = consts.tile([P, P], f32)
        nc.sync.dma_start(out=id_sb[:], in_=id_dram[:])

        # ---- burst x loads: up to TPB full 128-row tiles per DMA ----
        n_full = rows // P                       # full 128-row tiles (48)
        bursts = []                              # (first_tile, n_subtiles)
        t0 = 0
        while t0 < n_full:
            ntb = min(TPB, n_full - t0)
            bursts.append((t0, ntb))
            t0 += ntb

        xt_of = {}                               # tile idx -> (xt, col offset)

        def load_burst(t0, ntb):
            # One pool slot per burst, but issue per-tile sub-DMAs back to
            # back: the SDMA engines get deep descriptor queues (full memory-
            # level parallelism) while each tile's reduce only waits for its
            # own 1.6 MB.
            xt = xpool.tile([P, TPB * W], f32)
            for j in range(ntb):
                nc.sync.dma_start(out=xt[:, j * W:(j + 1) * W],
                                  in_=x[(t0 + j) * P:(t0 + j + 1) * P, :])
                xt_of[t0 + j] = (xt, j * W)

        TSPLIT = 12 * CH  # tail split point, on an m-slot boundary

        def load_tail():
            # two column sub-DMAs so the tail's reduces pipeline with its
            # own data arrival (this tile ends the stream; its latency is
            # exposed)
            xt = xtailp.tile([P, W], f32)
            r0 = n_full * P
            nc.sync.dma_start(out=xt[:rows - r0, 0:TSPLIT],
                              in_=x[r0:rows, 0:TSPLIT])
            nc.sync.dma_start(out=xt[:rows - r0, TSPLIT:W],
                              in_=x[r0:rows, TSPLIT:W])
            xt_of[n_full] = (xt, 0)

        next_burst = {t0: (t0, ntb) for (t0, ntb) in bursts}

        # output groups of GROUP tiles, but keep the ragged tail tile in its
        # own final group so the last (exposed) out-DMA is tiny
        group_of = {}
        g = 0
        while g < n_full:
            ge = min(g + GROUP, n_full)
            group_of[g] = (g, ge)
            g = ge
        if n_full < n_tiles:
            group_of[n_full] = (n_full, n_tiles)

        for t in range(n_tiles):
            r0 = t * P
            nb = min(P, rows - r0)

            if t in next_burst:
                load_burst(*next_burst[t])
            if t == n_full:
                load_tail()

            if t in group_of:
                g0, g1 = group_of[t]
                out_sb = opool.tile([P, GROUP * OUT_COLS], f32)
            ocols = slice((t - g0) * OUT_COLS, (t - g0 + 1) * OUT_COLS)

            if nb < P:
                # tail tile: zero the block so partitions [nb:] hold zeros
                nc.vector.memset(out_sb[:, ocols], 0.0)

            xt, xoff = xt_of.pop(t)
            st = spool.tile([P, S_DIM], f32)
            # permuted layout is uniformly [25 m-slots][128 channels], so the
            # whole channel-sum is ONE unit-stride reduce over [p, 25, 128]
            # (two for the tail tile, matching its two sub-DMAs)
            if nb < P:
                for c0, c1 in ((0, TSPLIT), (TSPLIT, W)):
                    src = xt[:nb, c0:c1].rearrange("p (m c) -> p m c", c=CH)
                    nc.vector.reduce_sum(out=st[:nb, c0 // CH:c1 // CH],
                                         in_=src, axis=mybir.AxisListType.X)
            else:
                src = xt[:nb, xoff:xoff + W].rearrange("p (m c) -> p m c",
                                                       c=CH)
                nc.vector.reduce_sum(out=st[:nb, :], in_=src,
                                     axis=mybir.AxisListType.X)

            pT = ps_t.tile([S_DIM, P], f32)
            nc.tensor.transpose(out=pT[:, :nb], in_=st[:nb, :],
                                identity=id_sb[:nb, :nb])
            sT = stpool.tile([S_DIM, P], f32)
            nc.scalar.copy(out=sT[:, :nb], in_=pT[:, :nb])

            po = ps_o.tile([P, OUT_COLS], f32)
            nc.tensor.matmul(out=po[:nb, :], lhsT=sT[:, :nb], rhs=a2_sb[:],
                             start=True, stop=True)
            nc.scalar.copy(out=out_sb[:nb, ocols], in_=po[:nb, :])

            if t == g1 - 1:
                # final (exposed) group goes via HWDGE: the x-ring is idle
                # by then and its first-byte latency is lower than SWDGE's
                eng = nc.sync if g1 == n_tiles else nc.gpsimd
                eng.dma_start(
                    out=out[:, g0:g1, :],
                    in_=out_sb[:, 0:(g1 - g0) * OUT_COLS].rearrange(
                        "p (t c) -> p t c", c=OUT_COLS))
    nc.finalize()
    return nc


_NC_CACHE = {}


def _get_nc(rows=ROWS):
    if rows not in _NC_CACHE:
        _NC_CACHE[rows] = _build_nc(rows)
    return _NC_CACHE[rows]


LAST_RESULTS = None  # BassKernelResults of the most recent kernel() call


def kernel(x_in: np.ndarray) -> np.ndarray:
    global LAST_RESULTS
    x = np.ascontiguousarray(np.asarray(x_in), dtype=np.float32)
    assert x.shape == (BATCH, DIM_IN), x.shape
    nc = _get_nc()
    xp = _permute(x)
    in_maps = [{"x": xp[c * ROWS:(c + 1) * ROWS]} for c in range(N_CORES)]
    res = run_bass_kernel_spmd(nc, in_maps, core_ids=list(range(N_CORES)))
    LAST_RESULTS = res
    outs = []
    for c in range(N_CORES):
        o = res.results[c]["out"]                       # [128, 49, 196]
        o = o.transpose(1, 0, 2).reshape(N_TILES * P, OUT_COLS)[:ROWS]
        outs.append(o)
    return np.concatenate(outs, axis=0).reshape(BATCH, 14, 14)
